# revision 1
# baseline (speedup 1.0000x reference)
"""Causal self-attention with token-shift LoRA modulation, Trainium2 Bass kernel.

Sharding: (batch x head-half). 8 cores = 4 batches x 2 half-head groups.
Core c handles batch b=c//2 and heads [8*(c%2), 8*(c%2)+8). Host sums the
2 partial [DIM, T] outputs per batch, adds proj_b, transposes to [B, T, C].

bf16 compute, fp32 PSUM accumulation. Chunk-causal attention: scores/exp/PV
column-restricted to the unmasked suffix; only the diagonal 128x128 block is
masked (tri-mask multiply on the Pool engine).

DVE elementwise ops are merged across contraction chunks / head chunks into
single wide 3D-AP instructions (free size 2048-4096) to amortize per-op
overhead; RoPE sin/cos are broadcast across head chunks with step-0 APs.

Scheduling: engine queues are in-order, so emission order sets overlap.
lu matmuls run ahead of the accumulating QKV matmuls; attention interleaves
the two heads of each 128-row chunk with depth-2 score/PV software
pipelining; the next q-window's projection work is woven as fillers into
the attention emission to cover PE stalls waiting on the exp stage.
"""

import numpy as np
import ml_dtypes

import concourse.bass as bass
import concourse.mybir as mybir
import concourse.tile as tile
from concourse.bass_utils import run_bass_kernel_spmd

B, T, DIM = 4, 1024, 1024
N_HEADS, HEAD_DIM, LORA = 16, 64, 16
N_CORES = 8
SL = DIM // 2                # 512 q/k/v dims per core (8 heads)
HPC = SL // HEAD_DIM         # heads per core = 8
OC = SL // 128               # 4 chunks of 128 rows (2 heads each)
NC8 = DIM // 128             # 8 contraction chunks
QT = 512                     # q token tile
NQT = T // QT                # 2
KC = T // 128                # 8 key chunks
F32 = mybir.dt.float32
F32R = mybir.dt.float32r
BF16 = mybir.dt.bfloat16
BF = ml_dtypes.bfloat16

_CACHE = {}


def build_program():
    nc = bass.Bass(trn_type="TRN2", target_bir_lowering=False, debug=False)

    xt = nc.dram_tensor("xt", [DIM, T], BF16, kind="ExternalInput")
    wq = nc.dram_tensor("wq", [DIM, SL], BF16, kind="ExternalInput")
    wk = nc.dram_tensor("wk", [DIM, SL], BF16, kind="ExternalInput")
    wv = nc.dram_tensor("wv", [DIM, SL], BF16, kind="ExternalInput")
    aaug = nc.dram_tensor("aaug", [DIM, 96], BF16, kind="ExternalInput")
    baug = nc.dram_tensor("baug", [96, DIM], BF16, kind="ExternalInput")
    pwt = nc.dram_tensor("pwt", [SL, DIM], BF16, kind="ExternalInput")
    cos4 = nc.dram_tensor("cos4", [128, T], BF16, kind="ExternalInput")
    sin4 = nc.dram_tensor("sin4", [128, T], BF16, kind="ExternalInput")
    tmask = nc.dram_tensor("tmask", [128, 128], BF16, kind="ExternalInput")
    ident = nc.dram_tensor("ident", [128, 128], BF16, kind="ExternalInput")
    onesrow = nc.dram_tensor("onesrow", [1, T], BF16, kind="ExternalInput")
    ones64d = nc.dram_tensor("ones64d", [1, HEAD_DIM], F32R, kind="ExternalInput")
    o = nc.dram_tensor("o", [DIM, T], BF16, kind="ExternalOutput")

    with tile.TileContext(nc) as tc:
        with (
            tc.tile_pool(name="consts", bufs=1) as consts,
            tc.tile_pool(name="xs", bufs=1) as xs_pool,
            tc.tile_pool(name="big", bufs=1) as big_pool,
            tc.tile_pool(name="qin", bufs=1) as qin_pool,
            tc.tile_pool(name="lup", bufs=1) as lu_pool,
            tc.tile_pool(name="mod", bufs=2) as mod_pool,
            tc.tile_pool(name="p", bufs=10) as p_pool,
            tc.tile_pool(name="small", bufs=4) as small_pool,
            tc.tile_pool(name="out", bufs=3) as out_pool,
            tc.tile_pool(name="psA", bufs=4, space="PSUM") as psA,
            tc.tile_pool(name="psB", bufs=4, space="PSUM") as psB,
        ):
            # ---- constants, DMA'd in first-use order ----
            # lora needs xs + a_sb first; lu needs b_sb; the big W loads can
            # trail behind the first ~10us of PE work instead of blocking it
            a_sb = consts.tile([128, NC8, 96], BF16, tag="a")
            nc.sync.dma_start(a_sb[:], aaug.rearrange("(k p) m -> p k m", p=128))
            b_sb = consts.tile([96, DIM], BF16, tag="b")
            nc.sync.dma_start(b_sb[:], baug[:])
            ones64 = consts.tile([1, HEAD_DIM], F32R, tag="ones64")
            nc.sync.dma_start(ones64[:], ones64d[:])
            xs = xs_pool.tile([128, NC8, T + 1], BF16, tag="xs")
            nc.vector.memset(xs[:, :, 0:1], 0.0)
            # chunked so the first lora matmul only waits on chunk 0
            for c8 in range(NC8):
                nc.sync.dma_start(
                    xs[:, c8, 1:T + 1], xt[c8 * 128:(c8 + 1) * 128, :])
            # s_aug ones rows feed the first lu matmuls -- keep their DMAs
            # ahead of the multi-MB weight loads in the queue
            s_aug = big_pool.tile([96, T], BF16, tag="saug")
            for i in range(3):
                # memset can't start at partition 16; DMA the ones row
                nc.sync.dma_start(s_aug[i * 32 + 16:i * 32 + 17, :], onesrow[:])
            w_sb = {}
            for name, dram in (("q", wq), ("k", wk), ("v", wv)):
                t_ = consts.tile([128, NC8, SL], BF16, tag=f"w{name}")
                nc.sync.dma_start(t_[:], dram.rearrange("(k p) m -> p k m", p=128))
                w_sb[name] = t_
            cos_sb = consts.tile([128, T], BF16, tag="cos")
            nc.sync.dma_start(cos_sb[:], cos4[:])
            sin_sb = consts.tile([128, T], BF16, tag="sin")
            nc.sync.dma_start(sin_sb[:], sin4[:])
            tm_sb = consts.tile([128, 128], BF16, tag="tmask")
            nc.sync.dma_start(tm_sb[:], tmask[:])
            id_sb = consts.tile([128, 128], BF16, tag="id")
            nc.sync.dma_start(id_sb[:], ident[:])
            pw_sb = consts.tile([128, OC, DIM], BF16, tag="pw")
            nc.sync.dma_start(pw_sb[:], pwt.rearrange("(k p) m -> p k m", p=128))

            q_sb = big_pool.tile([128, OC, T], BF16, tag="q")
            k_sb = big_pool.tile([128, OC, T], BF16, tag="k")
            v_aug = big_pool.tile([128, KC, HPC, HEAD_DIM + 1], BF16, tag="va")
            for h in range(HPC):
                nc.vector.memset(v_aug[:, :, h, HEAD_DIM:HEAD_DIM + 1], 1.0)
            outT = big_pool.tile([128, OC, T], BF16, tag="outT")

            NI = (("q", 0), ("k", 1), ("v", 2))

            # ---------- projection-pipeline steps for one q window ----------
            def proj_steps(qt):
                t0 = qt * QT
                cur = lambda c8: xs[:, c8, 1 + t0:1 + t0 + QT]
                cur_all = xs[:, :, 1 + t0:1 + t0 + QT]
                sft_all = xs[:, :, t0:t0 + QT]
                steps = []

                box = {}

                def lora_mm(c8):
                    def f():
                        if "s" not in box:
                            box["s"] = psA.tile([96, QT], F32, tag="ps",
                                                name=f"s{qt}")
                        nc.tensor.matmul(
                            box["s"][:], a_sb[:, c8, :], cur(c8),
                            start=(c8 == 0), stop=(c8 == NC8 - 1))
                    return f

                def tanh_step(i):
                    def f():
                        nc.scalar.activation(
                            s_aug[i * 32:i * 32 + LORA, t0:t0 + QT],
                            box["s"][i * 32:i * 32 + LORA, :],
                            mybir.ActivationFunctionType.Tanh)
                    return f

                def xxx_step():
                    def f():
                        box["xxx"] = big_pool.tile(
                            [128, NC8, QT], BF16, tag="xxx", name=f"xxx{qt}")
                        nc.gpsimd.tensor_sub(box["xxx"][:], sft_all, cur_all)
                    return f

                def lu_step(c8, n, i):
                    def f():
                        if n not in box:
                            box[n] = lu_pool.tile(
                                [128, NC8, QT], BF16, tag=f"lu{n}",
                                name=f"lu{n}{qt}")
                        ps_lu = psB.tile([128, QT], F32, tag="ps")
                        nc.tensor.matmul(
                            ps_lu[:],
                            b_sb[i * 32:i * 32 + 17, c8 * 128:(c8 + 1) * 128],
                            s_aug[i * 32:i * 32 + 17, t0:t0 + QT],
                            start=True, stop=True)
                        if c8 % 2 == 0:
                            nc.scalar.copy(box[n][:, c8, :], ps_lu[:])
                        else:
                            nc.vector.tensor_copy(box[n][:, c8, :], ps_lu[:])
                    return f

                def qin_step(n, i):
                    def f():
                        if "qin" not in box:
                            box["qin"] = qin_pool.tile(
                                [128, 3, NC8, QT], BF16, tag="qin",
                                name=f"qin{qt}")
                        dst = box["qin"][:, i, :, :]
                        nc.vector.tensor_mul(dst, box[n][:], box["xxx"][:])
                        nc.vector.tensor_add(dst, dst, cur_all)
                    return f

                def acc_mm(n, i, oc, c8):
                    def f():
                        key = ("acc", n, oc)
                        if key not in box:
                            box[key] = psA.tile(
                                [128, QT], F32, tag="ps",
                                name=f"acc{qt}{n}{oc}")
                        nc.tensor.matmul(
                            box[key][:],
                            w_sb[n][:, c8, oc * 128:(oc + 1) * 128],
                            box["qin"][:, i, c8, :],
                            start=(c8 == 0), stop=(c8 == NC8 - 1))
                    return f

                def rope(n, oc):
                    def f():
                        dst = q_sb if n == "q" else k_sb
                        acc = box.pop(("acc", n, oc))
                        sb = mod_pool.tile([128, QT], BF16, tag="ropesb")
                        nc.scalar.copy(sb[:], acc[:])
                        rot = mod_pool.tile([128, QT], BF16, tag="rot")
                        hh = HEAD_DIM // 2
                        for blk in range(4):
                            r0 = blk * hh
                            s0 = (blk ^ 1) * hh
                            # sin4 holds sign(dest block) at the SOURCE row
                            # base so both SBUF inputs share base partition
                            nc.vector.tensor_mul(
                                rot[r0:r0 + hh, :],
                                sb[s0:s0 + hh, :],
                                sin_sb[s0:s0 + hh, t0:t0 + QT])
                        nc.vector.tensor_mul(
                            sb[:], sb[:], cos_sb[:, t0:t0 + QT])
                        nc.vector.tensor_add(
                            dst[:, oc, t0:t0 + QT], sb[:], rot[:])
                    return f

                def vtrans(oc):
                    def f():
                        v_stage = mod_pool.tile([128, QT], BF16, tag="vst")
                        nc.scalar.copy(v_stage[:], box.pop(("acc", "v", oc))[:])
                        for j in range(QT // 128):
                            ki = qt * (QT // 128) + j
                            ps_t = psB.tile([128, 128], BF16, tag="ps")
                            with nc.allow_low_precision(reason="transpose"):
                                nc.tensor.transpose(
                                    ps_t[:], v_stage[:, j * 128:(j + 1) * 128],
                                    id_sb[:])
                            nc.scalar.copy(
                                v_aug[:, ki, 2 * oc:2 * oc + 2, 0:HEAD_DIM],
                                ps_t[:].rearrange("p (h d) -> p h d", h=2))
                    return f

                for c8 in range(NC8):
                    steps.append(lora_mm(c8))
                for i in range(3):
                    steps.append(tanh_step(i))
                steps.append(xxx_step())
                for c8 in range(NC8):
                    for n, i in NI:
                        steps.append(lu_step(c8, n, i))
                for n, i in NI:
                    steps.append(qin_step(n, i))
                for n, i in NI:
                    for oc in range(OC):
                        for c8 in range(NC8):
                            steps.append(acc_mm(n, i, oc, c8))
                        if n == "v":
                            steps.append(vtrans(oc))
                        else:
                            steps.append(rope(n, oc))
                return steps

            # ---------- output projection steps for one q window ----------
            def outproj_steps(qt):
                t0 = qt * QT
                steps = []

                def one(o8):
                    def f():
                        ps_f = psB.tile([128, QT], F32, tag="ps")
                        for cc in range(OC):
                            nc.tensor.matmul(
                                ps_f[:],
                                pw_sb[:, cc, o8 * 128:(o8 + 1) * 128],
                                outT[:, cc, t0:t0 + QT],
                                start=(cc == 0), stop=(cc == OC - 1))
                        f_sb = out_pool.tile([128, QT], BF16, tag="fsb")
                        if o8 % 2 == 0:
                            nc.scalar.copy(f_sb[:], ps_f[:])
                        else:
                            nc.vector.tensor_copy(f_sb[:], ps_f[:])
                        nc.sync.dma_start(
                            o[o8 * 128:(o8 + 1) * 128, t0:t0 + QT], f_sb[:])
                    return f

                for o8 in range(NC8):
                    steps.append(one(o8))
                return steps

            # ---------- attention for one q window, fillers woven ----------
            def emit_attention(qt, filler, group=2):
                t0 = qt * QT
                nki = (qt + 1) * (QT // 128)

                def fill(k=1):
                    for _ in range(k):
                        try:
                            next(filler)()
                        except StopIteration:
                            return

                pending_norm = []
                for g0 in range(0, HPC, group):
                    heads = tuple(range(g0, g0 + group))
                    ps_av = {}
                    for h in heads:
                        ps_av[h] = psA.tile([HEAD_DIM + 1, QT], F32, tag="ps",
                                            name=f"av{qt}{h}")
                    pbuf = {}

                    def sc_exp(h, ki):
                        oc, hb = h // 2, (h % 2) * HEAD_DIM
                        off = max(0, ki * 128 - t0)
                        ps_sc = psB.tile([128, QT], F32, tag="ps")
                        nc.tensor.matmul(
                            ps_sc[:, off:],
                            k_sb[hb:hb + HEAD_DIM, oc, ki * 128:(ki + 1) * 128],
                            q_sb[hb:hb + HEAD_DIM, oc, t0 + off:t0 + QT],
                            start=True, stop=True)
                        p = p_pool.tile([128, QT], BF16, tag="p")
                        nc.scalar.activation(
                            p[:, off:], ps_sc[:, off:],
                            mybir.ActivationFunctionType.Exp, scale=0.125)
                        if ki * 128 >= t0:
                            nc.gpsimd.tensor_mul(
                                p[:, off:off + 128], p[:, off:off + 128],
                                tm_sb[:])
                        pbuf[(h, ki)] = p

                    def pv(h, ki):
                        off = max(0, ki * 128 - t0)
                        p = pbuf.pop((h, ki))
                        nc.tensor.matmul(
                            ps_av[h][:, off:], v_aug[:, ki, h, :], p[:, off:],
                            start=(ki == 0), stop=(ki == nki - 1))

                    for h in heads:
                        sc_exp(h, 0)
                    # overlap the previous group's normalization chain with
                    # this group's pipeline warmup
                    for nf in pending_norm:
                        nf()
                        fill()
                    pending_norm = []
                    fill(2)
                    for ki in range(1, nki):
                        for h in heads:
                            sc_exp(h, ki)
                        fill(2)
                        for h in heads:
                            pv(h, ki - 1)
                        fill(2)
                    for h in heads:
                        pv(h, nki - 1)
                    fill(2)

                    # normalization: only one PSUM operand allowed per op,
                    # so stage the attention numerators through SBUF.
                    # split into engine-parallel stages: recip (DVE) + av
                    # copy (ACT) first, then broadcast (PE), then mul (DVE)
                    def norm_a(h, ps_av_h):
                        def f():
                            rinv = small_pool.tile([1, QT], F32R, tag="rinv",
                                                   name=f"rinv{qt}{h}")
                            with nc.allow_low_precision(
                                    reason="f32r = f32 bits"):
                                nc.vector.reciprocal(
                                    rinv[:],
                                    ps_av_h[HEAD_DIM:HEAD_DIM + 1, :])
                            av_sb = p_pool.tile([HEAD_DIM, QT], BF16,
                                                tag="avsb",
                                                name=f"avsb{qt}{h}")
                            nc.scalar.copy(av_sb[:], ps_av_h[0:HEAD_DIM, :])
                            return rinv, av_sb
                        return f

                    def norm_b(h, parts):
                        def f():
                            oc, hb = h // 2, (h % 2) * HEAD_DIM
                            rinv, av_sb = parts[h]
                            ps_bc = psB.tile([HEAD_DIM, QT], F32, tag="ps")
                            nc.tensor.matmul(
                                ps_bc[:], ones64[:], rinv[:],
                                start=True, stop=True)
                            nc.vector.tensor_mul(
                                outT[hb:hb + HEAD_DIM, oc, t0:t0 + QT],
                                av_sb[:], ps_bc[:])
                        return f

                    parts = {}

                    def stage_a(h, ps_av_h):
                        def f():
                            parts[h] = norm_a(h, ps_av_h)()
                        return f

                    pending_norm = [stage_a(h, ps_av[h]) for h in heads]
                    pending_norm += [norm_b(h, parts) for h in heads]
                for nf in pending_norm:
                    nf()
                    fill()

            # ---------- schedule ----------
            for step in proj_steps(0):
                step()
            filler1 = iter(proj_steps(1))
            emit_attention(0, filler1, group=2)
            for step in filler1:   # drain unconsumed projection work
                step()
            filler2 = iter(outproj_steps(0))
            emit_attention(1, filler2, group=4)
            for step in filler2:
                step()
            for step in outproj_steps(1):
                step()
    return nc


def _split_matmul_waits(nc):
    """Walrus limits sync-wait commands per instruction (1 for 4-byte-weight
    Matmult lowering, 2 for most other ops). Hoist excess waits onto
    preceding same-engine NoOps; engine program order preserves ordering."""
    for f in nc.m.functions:
        for blk in f.blocks:
            changed = False
            out = []
            for inst in blk.instructions:
                si = inst.sync_info
                nu = len(si.on_update) if si is not None and si.on_update else 0
                if isinstance(inst, (mybir.InstNoOp, mybir.InstDrain)):
                    keep = 1
                else:
                    keep = max(0, 2 - nu)
                if (si is not None and si.on_wait
                        and len(si.on_wait) > keep
                        and not isinstance(inst, mybir.InstNoOp)):
                    waits = list(si.on_wait)
                    extra, rest = waits[:-keep], waits[-keep:]
                    for j, w in enumerate(extra):
                        nop = mybir.InstNoOp(
                            name=f"{inst.name}-w{j}", engine=inst.engine)
                        nop.sync_info = mybir.SyncInfo(
                            on_wait=[w], on_update=[])
                        out.append(nop)
                    inst.sync_info = mybir.SyncInfo(
                        on_wait=rest, on_update=list(si.on_update or []))
                    changed = True
                out.append(inst)
            if changed:
                blk.instructions = out


def _prep_inputs(x, q_w, k_w, v_w, q_a, q_b, q_l, k_a, k_b, k_l,
                 v_a, v_b, v_l, proj_w, proj_b):
    aaug = np.zeros((DIM, 96), np.float32)
    for i, aa in enumerate((q_a, k_a, v_a)):
        aaug[:, i * 32:i * 32 + LORA] = aa.T
    baug = np.zeros((96, DIM), np.float32)
    for i, (bb, ll) in enumerate(((q_b, q_l), (k_b, k_l), (v_b, v_l))):
        baug[i * 32:i * 32 + LORA, :] = bb.T
        baug[i * 32 + LORA, :] = ll

    theta = 1.0 / (10000.0 ** (np.arange(0, HEAD_DIM, 2, dtype=np.float32)
                               / HEAD_DIM))
    pos = np.arange(T, dtype=np.float32)
    pt = pos[None, :] * theta[:, None]          # [32, T]
    cos1, sin1 = np.cos(pt), np.sin(pt)
    cos_h = np.concatenate([cos1, cos1], axis=0)     # [64, T]
    # sign of the DEST half-block, stored at the SOURCE half-block's rows:
    # dest rows 0:32 (xr' -= xi*sin) read source rows 32:64 -> -sin there.
    sin_h = np.concatenate([sin1, -sin1], axis=0)
    cos4 = np.tile(cos_h, (2, 1))                    # [128, T]
    sin4 = np.tile(sin_h, (2, 1))

    kk, qq = np.arange(128), np.arange(128)
    tmask = (qq[None, :] >= kk[:, None]).astype(np.float32)
    ident = np.eye(128, dtype=np.float32)

    in_maps = []
    for c in range(N_CORES):
        b, hh = c // 2, c % 2
        jsl = slice(hh * SL, (hh + 1) * SL)
        in_maps.append({
            "xt": np.ascontiguousarray(x[b].T).astype(BF),
            "wq": np.ascontiguousarray(q_w[jsl, :].T).astype(BF),
            "wk": np.ascontiguousarray(k_w[jsl, :].T).astype(BF),
            "wv": np.ascontiguousarray(v_w[jsl, :].T).astype(BF),
            "aaug": aaug.astype(BF),
            "baug": baug.astype(BF),
            "pwt": np.ascontiguousarray(proj_w[:, jsl].T).astype(BF),
            "cos4": cos4.astype(BF),
            "sin4": sin4.astype(BF),
            "tmask": tmask.astype(BF),
            "ident": ident.astype(BF),
            "onesrow": np.ones((1, T), np.float32).astype(BF),
            "ones64d": np.ones((1, HEAD_DIM), np.float32),
        })
    return in_maps


def kernel(**inputs):
    if "nc" not in _CACHE:
        nc = build_program()
        _split_matmul_waits(nc)
        _CACHE["nc"] = nc
    nc = _CACHE["nc"]
    in_maps = _prep_inputs(**inputs)
    res = run_bass_kernel_spmd(nc, in_maps, list(range(N_CORES)))
    out = np.empty((B, T, DIM), np.float32)
    bias = inputs["proj_b"][None, :]
    for b in range(B):
        acc = (res.results[2 * b]["o"].astype(np.float32)
               + res.results[2 * b + 1]["o"].astype(np.float32))
        out[b] = acc.T + bias
    return out

